# revision 33
# baseline (speedup 1.0000x reference)
"""BatchBlur: depthwise 15x15 conv with per-sample kernels, reflection pad 7.

x: (32, 3, 512, 512) f32, kernel: (32, 15, 15) f32 -> out (32, 3, 512, 512) f32.

Strategy: pure data parallel over batch, 4 samples (12 channel-images) per
core on 8 cores.  Host reflection-pads to 526 rows x 528 cols fp16.

Device formulation (triple-band, 4-way column tiling):
The 128x128 PE array is column-tiled into four 32-wide groups, one image
per group (tile_position=(0,32i)).  For each 28-output-row strip the rhs
tile holds the strip's 42 input rows three times, at column shifts
+0/+1/+2, so a banded stationary matrix A turns ONE accumulating matmul
into THREE horizontal taps for 28 output rows; five passes (rhs offset
3j) cover all 15 taps:  out[m,n] += sum_k A[k,j,m] * rhs[k, n+3j].
Per strip: 5 passes x 4 images but only 5 x 512 PE streaming cycles --
~1.85x fewer tensor cycles than the dual-band/2-image scheme.

Partition layout of each rhs tile:
  p  0..41   band2 (shift +2)  <- DVE copy from band0 (engine ops need
             32-aligned partition bases: src 64, dst 0; the +2-element
             shift keeps 4B alignment and a uint32 bitcast gets the 2x
             single-src DVE mode)
  p 42..63   band1 rows 0..21  (shift +1) |  host-staged together with
  p 64..105  band0 (shift 0)              |  band0: ONE 84-partition
  p106..125  band1 rows 22..41 (shift +1) |  coalesced load per unit
The A matrix rows are permuted to match.

Pipeline discipline (the load-bearing part, learned from traces):
 - DMA completion semaphores only reach their full 16-engine count
   ~5-8us after the transfer ends, so every DMA->consumer edge costs
   that latency.  Band 1 is therefore pre-staged on the host (2x HBM
   read on the input side) so the matmuls sit behind a SINGLE DMA edge,
   hidden by a 4-unit load prefetch; band 2 rides the DVE, whose
   completion semaphores are fast.  The first three units also ship
   band 2 from the host so nothing waits on the DVE at startup.
 - A DMA's lane-reuse guard (EVENT_SEMAPHORE on the previous user of
   its completion lane) parks the ISSUING engine's queue on that
   trickle, so stores live alone on the GpSimd ring (batched 4 strips
   per DMA), loads alone on the Sync ring, and the ACT queue carries
   only the PSUM->fp16 evictions; o_t and xin pools are deep enough to
   keep every reuse horizon past the trickle.
 - Quarter-col-tiled matmuls don't register in the HAM clock gate, so
   a full-row h0 heartbeat matmul per strip keeps the PE at 2.4GHz.

Outputs are stored raw as [3 groups, 19 strips, 128 partitions, 512]
(partition 32i+m = image 4g+i, row 28s+m; m=28..31 junk; strip 18 is the
r0=484 overlap strip whose rows 20..27 are the kept tail) and the host
gathers them back to (32, 3, 512, 512).
"""
import os
import sys

for _p in ("/opt/trn_rl_repo", "/root/.axon_site/_ro/trn_rl_repo"):
    if _p not in sys.path and os.path.isdir(_p):
        sys.path.insert(0, _p)

import numpy as np

import concourse.bass as bass
import concourse.mybir as mybir
import concourse.tile as tile
from concourse import bacc
from concourse.bass_utils import run_bass_kernel_spmd

L = 15           # blur kernel size
P = L // 2       # reflection pad
B, C, H, W = 32, 3, 512, 512
N_CORES = 8
BS = B // N_CORES            # samples per core
NIMG = BS * C                # channel images per core
HP, WP = H + 2 * P, W + 2 * P  # 526
WPH = WP + 2                 # host row pitch: 528 (two defined pad columns)
M_STRIP = 28                 # output rows per strip
KB = M_STRIP + L - 1         # 42-row band
KT = 126                     # matmul contraction (partitions 0..125)
NB = 3                       # bands (column shifts 0,1,2)
NP = 5                       # passes: dx = 3j+b, j=0..4
NG = NIMG // 4               # 3 groups of 4 column-tiled images
NS = 19                      # strips: 18 at r0=28s + final overlap at 484
R0_LAST = HP - KB            # 484
LOAD_AHEAD = 4               # loads issued this many units early on the ring
N_WARMUP = 40                # HAM warm-up matmuls (N=128 each)

F16 = mybir.dt.float16
F32 = mybir.dt.float32

_program_cache = None


def _build_program():
    nc = bacc.Bacc("TRN2", target_bir_lowering=False, debug=False)
    # host-staged band-0 AND band-1 rows in per-double-unit tile layout:
    # 84 rows = partitions 42..125 (band1 rows 0..21, band0 rows 0..41,
    # band1 rows 22..41); one 84-partition load per double unit means the
    # matmuls wait on a single DMA-completion edge (those semaphores only
    # fill ~5-8us after transfer end, so fewer edges >> faster pipeline).
    # du 9 is the single overlap strip in blocks 0..3.
    xq_d = nc.dram_tensor("xq", [NG, 10, 2 * KB, 8 * WPH], F16,
                          kind="ExternalInput").ap()
    # first three units ship ALL three bands from the host (no DVE edge
    # at startup, where the load-completion trickle is worst)
    xf_d = nc.dram_tensor("xf", [3, KT, 8 * WPH], F16,
                          kind="ExternalInput").ap()
    a_d = nc.dram_tensor("a", [BS, KT, NP, M_STRIP], F16,
                         kind="ExternalInput").ap()
    out_d = nc.dram_tensor("out", [NG, NS, 128, W], F16,
                           kind="ExternalOutput").ap()

    units = [(g, du) for g in range(NG) for du in range(10)]

    with tile.TileContext(nc) as tc:
        with (
            tc.tile_pool(name="aconst", bufs=1) as apool,
            tc.tile_pool(name="warm", bufs=1) as wpool,
            tc.tile_pool(name="xin", bufs=LOAD_AHEAD + 6) as xpool,
            tc.tile_pool(name="oout", bufs=12) as opool,
            tc.tile_pool(name="psum", bufs=7, space="PSUM") as psum,
            tc.tile_pool(name="psumw", bufs=1, space="PSUM") as psumw,
        ):
            tiles = {}

            def issue_load(k):
                g, du = units[k]
                t = xpool.tile([128, 2 * 4 * WPH], F16, tag="xdu",
                               name=f"x{k}")
                if k < 3:
                    nc.sync.dma_start(out=t[0:KT, :], in_=xf_d[k])
                elif du < 9:
                    nc.sync.dma_start(out=t[42:126, :], in_=xq_d[g, du])
                else:
                    nc.sync.dma_start(out=t[42:126, 0:4 * WPH],
                                      in_=xq_d[g, 9][:, 0:4 * WPH])
                tiles[k] = t

            def make_band2(t, nblk):
                w = nblk * WPH
                v0 = t[64:64 + KB, 0:w].rearrange("p (b c) -> p b c", c=WPH)
                v2 = t[0:KB, 0:w].rearrange("p (b c) -> p b c", c=WPH)
                nc.vector.tensor_copy(
                    out=v2[:, :, 0:524].bitcast(mybir.dt.uint32),
                    in_=v0[:, :, 2:526].bitcast(mybir.dt.uint32))

            def strip_mms(acc, x_t, a_t, samples, blk0):
                # 5 passes x 4 column-tiled images; K=126, M=28, N=512
                for j in range(NP):
                    for i in range(4):
                        col = (blk0 + i) * WPH + 3 * j
                        nc.tensor.matmul(
                            acc[32 * i:32 * i + M_STRIP],
                            a_t[samples[i]][:, j, :],
                            x_t[0:KT, col:col + W],
                            start=(j == 0),
                            stop=(j == NP - 1),
                            tile_position=(0, 32 * i),
                        )

            # HAM warm-up burst while the first loads are in flight
            wsrc = wpool.tile([128, 128], mybir.dt.bfloat16)
            nc.vector.memset(wsrc[:], 0.0)
            wacc = psumw.tile([64, 128], F32)
            for _ in range(N_WARMUP):
                nc.tensor.matmul(wacc[:], wsrc[:, :64], wsrc[:], start=True,
                                 stop=True)

            # prologue: first loads, then the A matrices on the GpSimd ring
            for k in range(LOAD_AHEAD):
                issue_load(k)
            a_t = [
                apool.tile([KT, NP, M_STRIP], F16, tag=f"a{s}",
                           name=f"a{s}")
                for s in range(BS)
            ]
            for s in range(BS):
                nc.scalar.dma_start(out=a_t[s][:], in_=a_d[s])

            # first unit's band2 (units 0..2 are fully host-staged)
            for k, (g, du) in enumerate(units):
                x_t = tiles.pop(k)
                samples = [(4 * g + i) // C for i in range(4)]
                nq = 2 if du < 9 else 1
                if k >= 3:
                    make_band2(x_t, 4 * nq)
                if k + LOAD_AHEAD < len(units):
                    issue_load(k + LOAD_AHEAD)
                if du % 2 == 0:
                    o_t = opool.tile([128, 4 * W], F16, tag="od4",
                                     name="od4")
                    o_slot = 0
                for q in range(nq):
                    acc = psum.tile([128, W], F32)
                    strip_mms(acc, x_t, a_t, samples, 4 * q)
                    # PSUM f32 -> SBUF fp16 on the ACT engine (issued
                    # before the heartbeats so it fires at MM-stop)
                    nc.scalar.copy(out=o_t[:, o_slot * W:(o_slot + 1) * W],
                                   in_=acc[:])
                    o_slot += 1
                    # full-row heartbeat: q-tiled matmuls are invisible
                    # to the HAM clock gate; this keeps K=8/8
                    nc.tensor.matmul(wacc[:], wsrc[:, :64], wsrc[:],
                                     start=True, stop=True)
                # stores ride GpSimd, batched 4 strips (the final store
                # covers 3): their lane-reuse guards park on the ~6us
                # DMA-completion trickle, so fewer guards per group keeps
                # the store stream ahead of the o_t reuse horizon, and
                # they must not queue ahead of casts (ACT) / loads (Sync)
                if du in (1, 3, 5, 7) or du == 9:
                    s0 = 2 * (du - 1) if du < 9 else 16
                    nw = 4 if du < 9 else 3
                    nc.gpsimd.dma_start(
                        out=out_d[g, s0:s0 + nw].rearrange("q p c -> p q c"),
                        in_=o_t[:, 0:nw * W].rearrange("p (q c) -> p q c",
                                                       c=W))
    nc.compile()
    return nc


def prepare_in_maps(x: np.ndarray, kern: np.ndarray) -> list:
    # host-side reflection pad, fp16, rows padded to WPH with zero columns
    xpc = np.pad(x, ((0, 0), (0, 0), (P, P), (P, P)), mode="reflect")
    xp = np.zeros((B * C, HP, WPH), dtype=np.float16)
    xp[:, :, :WP] = xpc.reshape(B * C, HP, WP).astype(np.float16)

    # band-0 + band-1 rows restaged per double-unit: [core, g, du, 84, 8,
    # WPH]; staged row j = tile partition 42+j: j 0..21 band1 rows 0..21,
    # j 22..63 band0, j 64..83 band1 rows 22..41.  band1 is pre-shifted by
    # one column on the host (its col 527 stays zero -- never streamed).
    # Block q*4+i = strip 56*du+28*q of image 4g+i (du 9: overlap strip
    # at r0=484 in blocks 0..3).
    xv = xp.reshape(N_CORES, NG, 4, HP, WPH)
    xq = np.zeros((N_CORES, NG, 10, 2 * KB, 8, WPH), dtype=np.float16)

    def stage(du, blk, r0):
        rows = xv[:, :, :, r0:r0 + KB, :].transpose(0, 1, 3, 2, 4)
        xq[:, :, du, 22:64, blk:blk + 4, :] = rows
        xq[:, :, du, 0:22, blk:blk + 4, 0:WPH - 1] = rows[:, :, 0:22, :, 1:]
        xq[:, :, du, 64:84, blk:blk + 4, 0:WPH - 1] = rows[:, :, 22:, :, 1:]

    for du in range(9):
        for q in range(2):
            stage(du, 4 * q, 56 * du + 28 * q)
    stage(9, 0, R0_LAST)

    # triple-band stationary matrices; partition layout (r = band row):
    #   k = r       band2 (dx=3j+2)
    #   k = 64+r    band0 (dx=3j)
    #   k = 42+r (r<22) / 84+r (r>=22)  band1 (dx=3j+1)
    kern16 = kern.astype(np.float16)
    a_all = np.zeros((B, KT, NP, M_STRIP), dtype=np.float16)
    m_idx = np.arange(M_STRIP)
    for dy in range(L):
        r = m_idx + dy
        a_all[:, r, :, m_idx] = kern16[:, dy, 2::NB]
        a_all[:, 64 + r, :, m_idx] = kern16[:, dy, 0::NB]
        k1 = np.where(r < 22, 42 + r, 84 + r)
        a_all[:, k1, :, m_idx] = kern16[:, dy, 1::NB]

    # fully-staged first three units (g=0, du 0..2): band2 rows 0..41 at
    # partitions 0..41 (pre-shifted +2), then the xq rows at 42..125
    xf = np.zeros((N_CORES, 3, KT, 8, WPH), dtype=np.float16)
    for k in range(3):
        for q in range(2):
            r0 = 56 * k + 28 * q
            rt = xv[:, 0, :, r0:r0 + KB, :].transpose(0, 2, 1, 3)
            xf[:, k, 0:KB, 4 * q:4 * q + 4, 0:WPH - 2] = rt[..., 2:]
        xf[:, k, KB:KT] = xq[:, 0, k, 0:KT - KB]

    return [
        {
            "xq": xq[c].reshape(NG, 10, 2 * KB, 8 * WPH),
            "xf": xf[c].reshape(3, KT, 8 * WPH),
            "a": a_all[c * BS:(c + 1) * BS],
        }
        for c in range(N_CORES)
    ]


def kernel(x: np.ndarray, kernel: np.ndarray) -> np.ndarray:
    global _program_cache
    x = np.asarray(x, dtype=np.float32)
    kern = np.asarray(kernel, dtype=np.float32)

    in_maps = prepare_in_maps(x, kern)
    if _program_cache is None:
        _program_cache = _build_program()
    nc = _program_cache

    res = run_bass_kernel_spmd(nc, in_maps, core_ids=list(range(N_CORES)))
    outs = []
    for r in res.results:
        o = r["out"].reshape(NG, NS, 4, 32, W)[:, :, :, :M_STRIP, :]
        o = o.transpose(0, 2, 1, 3, 4)          # [g, i, s, m, c]
        body = o[:, :, :18].reshape(NIMG, 18 * M_STRIP, W)
        tail = o[:, :, 18, 504 - R0_LAST:, :].reshape(NIMG, H - 18 * M_STRIP,
                                                      W)
        outs.append(np.concatenate([body, tail], axis=1))
    out = np.concatenate(outs, axis=0)
    return out.reshape(B, C, H, W).astype(np.float32)


# revision 34
# speedup vs baseline: 1.0560x; 1.0560x over previous
"""BatchBlur: depthwise 15x15 conv with per-sample kernels, reflection pad 7.

x: (32, 3, 512, 512) f32, kernel: (32, 15, 15) f32 -> out (32, 3, 512, 512) f32.

Strategy: pure data parallel over batch, 4 samples (12 channel-images) per
core on 8 cores.  Host reflection-pads to 526 rows x 528 cols fp16.

Device formulation (triple-band, 4-way column tiling):
The 128x128 PE array is column-tiled into four 32-wide groups, one image
per group (tile_position=(0,32i)).  For each 28-output-row strip the rhs
tile holds the strip's 42 input rows three times, at column shifts
+0/+1/+2, so a banded stationary matrix A turns ONE accumulating matmul
into THREE horizontal taps for 28 output rows; five passes (rhs offset
3j) cover all 15 taps:  out[m,n] += sum_k A[k,j,m] * rhs[k, n+3j].
Per strip: 5 passes x 4 images but only 5 x 512 PE streaming cycles --
~1.85x fewer tensor cycles than the dual-band/2-image scheme.

Partition layout of each rhs tile:
  p  0..41   band2 (shift +2)  <- DVE copy from band0 (engine ops need
             32-aligned partition bases: src 64, dst 0; the +2-element
             shift keeps 4B alignment and a uint32 bitcast gets the 2x
             single-src DVE mode)
  p 42..63   band1 rows 0..21  (shift +1) |  host-staged together with
  p 64..105  band0 (shift 0)              |  band0: ONE 84-partition
  p106..125  band1 rows 22..41 (shift +1) |  coalesced load per unit
The A matrix rows are permuted to match.

Pipeline discipline (the load-bearing part, learned from traces):
 - DMA completion semaphores only reach their full 16-engine count
   ~5-8us after the transfer ends, so every DMA->consumer edge costs
   that latency.  Band 1 is therefore pre-staged on the host (2x HBM
   read on the input side) so the matmuls sit behind a SINGLE DMA edge,
   hidden by a 4-unit load prefetch; band 2 rides the DVE, whose
   completion semaphores are fast.  The first three units also ship
   band 2 from the host so nothing waits on the DVE at startup.
 - A DMA's lane-reuse guard (EVENT_SEMAPHORE on the previous user of
   its completion lane) parks the ISSUING engine's queue on that
   trickle, so stores live alone on the GpSimd ring (batched 4 strips
   per DMA), loads alone on the Sync ring, and the ACT queue carries
   only the PSUM->fp16 evictions; o_t and xin pools are deep enough to
   keep every reuse horizon past the trickle.
 - Quarter-col-tiled matmuls don't register in the HAM clock gate, so
   a full-row h0 heartbeat matmul per strip keeps the PE at 2.4GHz.

Outputs are stored raw as [3 groups, 19 strips, 128 partitions, 512]
(partition 32i+m = image 4g+i, row 28s+m; m=28..31 junk; strip 18 is the
r0=484 overlap strip whose rows 20..27 are the kept tail) and the host
gathers them back to (32, 3, 512, 512).
"""
import os
import sys

for _p in ("/opt/trn_rl_repo", "/root/.axon_site/_ro/trn_rl_repo"):
    if _p not in sys.path and os.path.isdir(_p):
        sys.path.insert(0, _p)

import numpy as np

import concourse.bass as bass
import concourse.mybir as mybir
import concourse.tile as tile
from concourse import bacc
from concourse.bass_utils import run_bass_kernel_spmd

L = 15           # blur kernel size
P = L // 2       # reflection pad
B, C, H, W = 32, 3, 512, 512
N_CORES = 8
BS = B // N_CORES            # samples per core
NIMG = BS * C                # channel images per core
HP, WP = H + 2 * P, W + 2 * P  # 526
WPH = WP + 2                 # host row pitch: 528 (two defined pad columns)
M_STRIP = 28                 # output rows per strip
KB = M_STRIP + L - 1         # 42-row band
KT = 126                     # matmul contraction (partitions 0..125)
NB = 3                       # bands (column shifts 0,1,2)
NP = 5                       # passes: dx = 3j+b, j=0..4
NG = NIMG // 4               # 3 groups of 4 column-tiled images
NS = 19                      # strips: 18 at r0=28s + final overlap at 484
R0_LAST = HP - KB            # 484
LOAD_AHEAD = 5               # loads issued this many units early on the ring
N_WARMUP = 40                # HAM warm-up matmuls (N=128 each)

F16 = mybir.dt.float16
F32 = mybir.dt.float32

_program_cache = None


def _build_program():
    nc = bacc.Bacc("TRN2", target_bir_lowering=False, debug=False)
    # host-staged band-0 AND band-1 rows in per-double-unit tile layout:
    # 84 rows = partitions 42..125 (band1 rows 0..21, band0 rows 0..41,
    # band1 rows 22..41); one 84-partition load per double unit means the
    # matmuls wait on a single DMA-completion edge (those semaphores only
    # fill ~5-8us after transfer end, so fewer edges >> faster pipeline).
    # du 9 is the single overlap strip in blocks 0..3.
    xq_d = nc.dram_tensor("xq", [NG, 10, 2 * KB, 8 * WPH], F16,
                          kind="ExternalInput").ap()
    # first three units ship ALL three bands from the host (no DVE edge
    # at startup, where the load-completion trickle is worst)
    xf_d = nc.dram_tensor("xf", [3, KT, 8 * WPH], F16,
                          kind="ExternalInput").ap()
    a_d = nc.dram_tensor("a", [BS, KT, NP, M_STRIP], F16,
                         kind="ExternalInput").ap()
    out_d = nc.dram_tensor("out", [NG, NS, 128, W], F16,
                           kind="ExternalOutput").ap()

    units = [(g, du) for g in range(NG) for du in range(10)]

    with tile.TileContext(nc) as tc:
        with (
            tc.tile_pool(name="aconst", bufs=1) as apool,
            tc.tile_pool(name="warm", bufs=1) as wpool,
            tc.tile_pool(name="xin", bufs=LOAD_AHEAD + 6) as xpool,
            tc.tile_pool(name="oout", bufs=14) as opool,
            tc.tile_pool(name="psum", bufs=7, space="PSUM") as psum,
            tc.tile_pool(name="psumw", bufs=1, space="PSUM") as psumw,
        ):
            tiles = {}

            def issue_load(k):
                g, du = units[k]
                t = xpool.tile([128, 2 * 4 * WPH], F16, tag="xdu",
                               name=f"x{k}")
                if k < 3:
                    nc.sync.dma_start(out=t[0:KT, :], in_=xf_d[k])
                elif du < 9:
                    nc.sync.dma_start(out=t[42:126, :], in_=xq_d[g, du])
                else:
                    nc.sync.dma_start(out=t[42:126, 0:4 * WPH],
                                      in_=xq_d[g, 9][:, 0:4 * WPH])
                tiles[k] = t

            def make_band2(t, nblk):
                w = nblk * WPH
                v0 = t[64:64 + KB, 0:w].rearrange("p (b c) -> p b c", c=WPH)
                v2 = t[0:KB, 0:w].rearrange("p (b c) -> p b c", c=WPH)
                nc.vector.tensor_copy(
                    out=v2[:, :, 0:524].bitcast(mybir.dt.uint32),
                    in_=v0[:, :, 2:526].bitcast(mybir.dt.uint32))

            def strip_mms(acc, x_t, a_t, samples, blk0):
                # 5 passes x 4 column-tiled images; K=126, M=28, N=512
                for j in range(NP):
                    for i in range(4):
                        col = (blk0 + i) * WPH + 3 * j
                        nc.tensor.matmul(
                            acc[32 * i:32 * i + M_STRIP],
                            a_t[samples[i]][:, j, :],
                            x_t[0:KT, col:col + W],
                            start=(j == 0),
                            stop=(j == NP - 1),
                            tile_position=(0, 32 * i),
                        )

            # HAM warm-up burst while the first loads are in flight
            wsrc = wpool.tile([128, 128], mybir.dt.bfloat16)
            nc.vector.memset(wsrc[:], 0.0)
            wacc = psumw.tile([64, 128], F32)
            for _ in range(N_WARMUP):
                nc.tensor.matmul(wacc[:], wsrc[:, :64], wsrc[:], start=True,
                                 stop=True)

            # prologue: first loads, then the A matrices on the GpSimd ring
            for k in range(LOAD_AHEAD):
                issue_load(k)
            a_t = [
                apool.tile([KT, NP, M_STRIP], F16, tag=f"a{s}",
                           name=f"a{s}")
                for s in range(BS)
            ]
            for s in range(BS):
                nc.scalar.dma_start(out=a_t[s][:], in_=a_d[s])

            # first unit's band2 (units 0..2 are fully host-staged)
            for k, (g, du) in enumerate(units):
                x_t = tiles.pop(k)
                samples = [(4 * g + i) // C for i in range(4)]
                nq = 2 if du < 9 else 1
                if k >= 3:
                    make_band2(x_t, 4 * nq)
                if k + LOAD_AHEAD < len(units):
                    issue_load(k + LOAD_AHEAD)
                if du % 2 == 0:
                    o_t = opool.tile([128, 4 * W], F16, tag="od4",
                                     name="od4")
                    o_slot = 0
                for q in range(nq):
                    acc = psum.tile([128, W], F32)
                    strip_mms(acc, x_t, a_t, samples, 4 * q)
                    # PSUM f32 -> SBUF fp16 on the ACT engine (issued
                    # before the heartbeats so it fires at MM-stop)
                    nc.scalar.copy(out=o_t[:, o_slot * W:(o_slot + 1) * W],
                                   in_=acc[:])
                    o_slot += 1
                    # full-row heartbeat: q-tiled matmuls are invisible
                    # to the HAM clock gate; this keeps K=8/8
                    nc.tensor.matmul(wacc[:], wsrc[:, :64], wsrc[:],
                                     start=True, stop=True)
                # stores ride GpSimd, batched 4 strips (the final store
                # covers 3): their lane-reuse guards park on the ~6us
                # DMA-completion trickle, so fewer guards per group keeps
                # the store stream ahead of the o_t reuse horizon, and
                # they must not queue ahead of casts (ACT) / loads (Sync)
                if du in (1, 3, 5, 7) or du == 9:
                    s0 = 2 * (du - 1) if du < 9 else 16
                    nw = 4 if du < 9 else 3
                    nc.gpsimd.dma_start(
                        out=out_d[g, s0:s0 + nw].rearrange("q p c -> p q c"),
                        in_=o_t[:, 0:nw * W].rearrange("p (q c) -> p q c",
                                                       c=W))
    nc.compile()
    return nc


def prepare_in_maps(x: np.ndarray, kern: np.ndarray) -> list:
    # host-side reflection pad, fp16, rows padded to WPH with zero columns
    xpc = np.pad(x, ((0, 0), (0, 0), (P, P), (P, P)), mode="reflect")
    xp = np.zeros((B * C, HP, WPH), dtype=np.float16)
    xp[:, :, :WP] = xpc.reshape(B * C, HP, WP).astype(np.float16)

    # band-0 + band-1 rows restaged per double-unit: [core, g, du, 84, 8,
    # WPH]; staged row j = tile partition 42+j: j 0..21 band1 rows 0..21,
    # j 22..63 band0, j 64..83 band1 rows 22..41.  band1 is pre-shifted by
    # one column on the host (its col 527 stays zero -- never streamed).
    # Block q*4+i = strip 56*du+28*q of image 4g+i (du 9: overlap strip
    # at r0=484 in blocks 0..3).
    xv = xp.reshape(N_CORES, NG, 4, HP, WPH)
    xq = np.zeros((N_CORES, NG, 10, 2 * KB, 8, WPH), dtype=np.float16)

    def stage(du, blk, r0):
        rows = xv[:, :, :, r0:r0 + KB, :].transpose(0, 1, 3, 2, 4)
        xq[:, :, du, 22:64, blk:blk + 4, :] = rows
        xq[:, :, du, 0:22, blk:blk + 4, 0:WPH - 1] = rows[:, :, 0:22, :, 1:]
        xq[:, :, du, 64:84, blk:blk + 4, 0:WPH - 1] = rows[:, :, 22:, :, 1:]

    for du in range(9):
        for q in range(2):
            stage(du, 4 * q, 56 * du + 28 * q)
    stage(9, 0, R0_LAST)

    # triple-band stationary matrices; partition layout (r = band row):
    #   k = r       band2 (dx=3j+2)
    #   k = 64+r    band0 (dx=3j)
    #   k = 42+r (r<22) / 84+r (r>=22)  band1 (dx=3j+1)
    kern16 = kern.astype(np.float16)
    a_all = np.zeros((B, KT, NP, M_STRIP), dtype=np.float16)
    m_idx = np.arange(M_STRIP)
    for dy in range(L):
        r = m_idx + dy
        a_all[:, r, :, m_idx] = kern16[:, dy, 2::NB]
        a_all[:, 64 + r, :, m_idx] = kern16[:, dy, 0::NB]
        k1 = np.where(r < 22, 42 + r, 84 + r)
        a_all[:, k1, :, m_idx] = kern16[:, dy, 1::NB]

    # fully-staged first three units (g=0, du 0..2): band2 rows 0..41 at
    # partitions 0..41 (pre-shifted +2), then the xq rows at 42..125
    xf = np.zeros((N_CORES, 3, KT, 8, WPH), dtype=np.float16)
    for k in range(3):
        for q in range(2):
            r0 = 56 * k + 28 * q
            rt = xv[:, 0, :, r0:r0 + KB, :].transpose(0, 2, 1, 3)
            xf[:, k, 0:KB, 4 * q:4 * q + 4, 0:WPH - 2] = rt[..., 2:]
        xf[:, k, KB:KT] = xq[:, 0, k, 0:KT - KB]

    return [
        {
            "xq": xq[c].reshape(NG, 10, 2 * KB, 8 * WPH),
            "xf": xf[c].reshape(3, KT, 8 * WPH),
            "a": a_all[c * BS:(c + 1) * BS],
        }
        for c in range(N_CORES)
    ]


def kernel(x: np.ndarray, kernel: np.ndarray) -> np.ndarray:
    global _program_cache
    x = np.asarray(x, dtype=np.float32)
    kern = np.asarray(kernel, dtype=np.float32)

    in_maps = prepare_in_maps(x, kern)
    if _program_cache is None:
        _program_cache = _build_program()
    nc = _program_cache

    res = run_bass_kernel_spmd(nc, in_maps, core_ids=list(range(N_CORES)))
    outs = []
    for r in res.results:
        o = r["out"].reshape(NG, NS, 4, 32, W)[:, :, :, :M_STRIP, :]
        o = o.transpose(0, 2, 1, 3, 4)          # [g, i, s, m, c]
        body = o[:, :, :18].reshape(NIMG, 18 * M_STRIP, W)
        tail = o[:, :, 18, 504 - R0_LAST:, :].reshape(NIMG, H - 18 * M_STRIP,
                                                      W)
        outs.append(np.concatenate([body, tail], axis=1))
    out = np.concatenate(outs, axis=0)
    return out.reshape(B, C, H, W).astype(np.float32)
